# revision 45
# baseline (speedup 1.0000x reference)
"""Distributed masked multi-head self-attention for Trainium2 (8 NeuronCores).

Problem: x:[2,2048,1024], mask:[2,2048], Wq:[1024,1024], Wkv:[1024,2048],
Wo:[1024,1024]  ->  out:[2,2048,1024]  (fp32)

Strategy (single SPMD launch, one NEFF on 8 cores), v2 -- optimized for
SINGLE-LAUNCH exec time (the graded metric), not steady-state throughput:

  Head parallel: core c owns heads {2c, 2c+1} (128 contiguous columns of
  q/k/v).  Keys are COMPACTED on the host: only mask-valid key positions
  are shipped (padded up to a multiple of 128, KCAP).  Padding slots
  carry a -30000 bias so exp()=0.

  Per (b, jh) query chunk of 1024 the attention runs in "scores
  transposed" [key, query] layout:
      simT = kT_h as lhsT, qT as rhs -> [128 keys, 1024 q] in PSUM
      P    = exp(scale*simT + pad_bias)                     (ScalarE)
      pv[128,q] += [v_h | ones*64]^T @ P
  The 64 ones-columns replicate the softmax denominator across PSUM
  partitions 64:128 (extra lhsT columns are free: matmul time only
  depends on the moving free size).

  v3: the attention t-loop and chunk boundaries are software-pipelined
  at EMISSION level: QK(t+1) is emitted between exp(t) and PV(t), and the
  next chunk's QK(0) at the last iteration, so the in-order-biased PE
  dequeues QK the moment exp frees its PSUM sim bank.  This closes a
  ~600ns/key-tile bubble in the exp->QK->exp chain (PSUM's 8 banks only
  allow double-buffered sim tiles, so the chain latency otherwise trails
  ScalarE's back-to-back exp rate).  Changes are emission-order only --
  instructions, operands and numerics are identical to v2.

  v2 structural changes vs v1 (one big end-of-kernel AllToAll):
  1. FOUR per-chunk AllToAlls.  Output rows are row-interleaved: core c
     owns rows c*128..c*128+128 of EVERY 1024-row chunk, so each chunk's
     collective gives every core a full [1024 cols x 128 rows] slab and
     its out-projection slice can run as soon as that chunk's collective
     lands -- the first three collectives and out-proj slices hide under
     later attention chunks; only the last chunk's ~21us collective is
     exposed (v1 exposed a 41us collective plus the whole out-proj).
  2. Streamed head: input DMAs are issued in consumption order (wk/wv/wq,
     xk[b0], xT[b0] first half, mb, xT[b0] second half, xk[b1],
     xT[b1]h0, wo, xT[b1]h1) and k/v are projected before q, so the
     first attention chunk starts ~15us earlier.  A long PE warmup chain
     (~10us of dependency-free matmuls) spans the initial DMA wait so the
     tensor engine is at full DVFS p-state when real work arrives.
  3. Projections for batch 1 and the q tail of batch 0 are deferred into
     attention t-loop slots ("fillers") scheduled against DMA arrival
     times, keeping the PE stream dense without stalling on un-landed
     tiles.
  4. Softmax normalize = DVE reciprocal + Pool (gpsimd) multiply
     (fp32 PSUM -> bf16 SBUF), spreading engine load; the a2a staging DMA
     fires per 64-partition half as soon as it is normalized.
  5. A chain of dependency-free dummy matmuls bridges the PE gap while
     the last collective is in flight, keeping the p-state ramp warm for
     the final out-projection slice.

Precision: x/xk/Wq/Wk/Wv/q/k/P/v/att/Wo bf16; PSUM accumulation, softmax
denominators and output fp32 (measured rel err ~4.6e-3, tol 2e-2).
"""

import os
import sys

import numpy as np

for _p in ("/opt/trn_rl_repo",):
    if _p not in sys.path and os.path.isdir(_p):
        sys.path.append(_p)

import concourse.bass as bass
from concourse import bacc
import concourse.mybir as mybir
import concourse.tile as tile
from contextlib import ExitStack

# ----- problem constants (hardcoded; kernel.py must be self-contained) -----
B, N, DIM, H, DH = 2, 2048, 1024, 16, 64
DI = H * DH                       # 1024
NCORES = 8
HPC = H // NCORES                 # 2 heads per core
CW = HPC * DH                     # 128 att columns per core
RPC = B * N // NCORES             # 512 output rows per core
SCALE = DH ** -0.5
MASK_NEG = -30000.0               # exp(scale*s + MASK_NEG) == 0 in fp32

P = 128                           # partitions
KT = DIM // P                     # 8 contraction tiles for projections
QW = 1024                         # query chunk width
NQH = N // QW                     # 2 query chunks per batch
NCHUNK = B * NQH                  # 4 chunks / collectives
FP32 = mybir.dt.float32
BF16 = mybir.dt.bfloat16

NWARM = 25                        # PE warmup matmuls (~ends when xk lands)
ND1 = 40                          # keep-warm matmuls before out-proj c2
ND2 = 82                          # keep-warm matmuls during last collective


def build_program(reps=1, kcaps=(1152, 1152), collective=True):
    NKTS = [k // P for k in kcaps]   # key tiles per batch
    NKT = max(NKTS)
    kcap = max(kcaps)
    nc = bacc.Bacc(None, target_bir_lowering=False, num_devices=NCORES)

    warm = nc.dram_tensor("warm", [1, 4], FP32, kind="ExternalOutput")
    xt = nc.dram_tensor("xt", [B, DIM, N], BF16, kind="ExternalInput")
    xk = nc.dram_tensor("xk", [B, DIM, kcap], BF16, kind="ExternalInput")
    # weights pre-transposed on host to [P, KT, *] so each DMA row is one
    # long contiguous descriptor (2KB+) instead of 256B strips (2x penalty)
    wq = nc.dram_tensor("wq", [P, KT, CW], BF16, kind="ExternalInput")
    wk = nc.dram_tensor("wk", [P, KT, CW], BF16, kind="ExternalInput")
    wv = nc.dram_tensor("wv", [P, KT, CW], BF16, kind="ExternalInput")
    wo = nc.dram_tensor("wo", [P, KT, DIM], BF16, kind="ExternalInput")
    mb = nc.dram_tensor("mb", [P, B * NKT], FP32, kind="ExternalInput")
    # out[c, r, :]: chunk c (= 2b+jh), this core's row r within its
    # 128-row interleaved slice of that chunk
    out = nc.dram_tensor("out", [NCHUNK, P, DIM], FP32, kind="ExternalOutput")

    with tile.TileContext(nc) as tc, ExitStack() as ctx, \
            nc.allow_low_precision(reason="bf16 pipeline; psum accum stays fp32"):
        const = ctx.enter_context(tc.tile_pool(name="const", bufs=1))
        wts = ctx.enter_context(tc.tile_pool(name="wts", bufs=1))
        wop = ctx.enter_context(tc.tile_pool(name="wop", bufs=1))
        xtp = ctx.enter_context(tc.tile_pool(name="xtp", bufs=2 * B * KT))
        xkp = ctx.enter_context(tc.tile_pool(name="xkp", bufs=B * KT))
        qkp = ctx.enter_context(tc.tile_pool(name="qkp", bufs=2))
        vsp = ctx.enter_context(tc.tile_pool(name="vsp", bufs=2))
        pp = ctx.enter_context(tc.tile_pool(name="pp", bufs=6))
        sml = ctx.enter_context(tc.tile_pool(name="sml", bufs=4))
        atb = ctx.enter_context(tc.tile_pool(name="atb", bufs=3))
        a2p = ctx.enter_context(tc.tile_pool(name="a2p", bufs=4))
        osp = ctx.enter_context(tc.tile_pool(name="osp", bufs=2))
        ps = ctx.enter_context(tc.tile_pool(name="ps", bufs=1, space="PSUM"))
        dram = ctx.enter_context(tc.tile_pool(name="dram", bufs=1, space="DRAM"))

        a2a_in = [dram.tile([NCORES * P, P], BF16, tag=f"a2ai{c}",
                            name=f"a2a_in{c}") for c in range(NCHUNK)]
        a2a_out = [dram.tile([NCORES * P, P], BF16, tag=f"a2ao{c}",
                             name=f"a2a_out{c}") for c in range(NCHUNK)]

        # ---- weights first on the sync queue (k-proj needs wk earliest) ----
        wk_sb = wts.tile([P, KT, CW], BF16, tag="wk")
        wv_sb = wts.tile([P, KT, CW], BF16, tag="wv")
        wq_sb = wts.tile([P, KT, CW], BF16, tag="wq")
        for w_sb, w in ((wk_sb, wk), (wv_sb, wv), (wq_sb, wq)):
            nc.sync.dma_start(out=w_sb[:], in_=w[:])
        wo_sb = wop.tile([P, KT, DIM], BF16, tag="wo")   # DMA issued late
        mb_sb = const.tile([P, B * NKT], FP32, tag="mb")  # DMA issued on scalar q

        # ---- PE warmup chain + Exp act-table preload (both off critical deps)
        wz = const.tile([P, 512], BF16, tag="wz")
        nc.vector.memset(wz[:], 0.0)
        nc.scalar.dma_start(out=mb_sb[:], in_=mb[:])
        wexp = const.tile([1, 4], FP32, tag="wexp")
        nc.scalar.activation(wexp[:], wz[0:1, 0:4],
                             mybir.ActivationFunctionType.Exp,
                             bias=0.0, scale=1.0)
        wps = ps.tile([64, 512], FP32, tag="pvA", name="warmps")
        for i in range(NWARM):
            nc.tensor.matmul(wps[:], wz[:, 0:64], wz[:],
                             start=(i == 0), stop=(i == NWARM - 1))
        wsb = const.tile([1, 4], FP32, tag="wsb")
        nc.vector.tensor_copy(wsb[:], wps[0:1, 0:4])
        nc.sync.dma_start(out=warm[:], in_=wsb[:])

        qT = {}
        kT = {}
        v_sb = {}

        tagc = [0]

        def next_tag(tags):
            tagc[0] += 1
            return tags[tagc[0] % len(tags)]

        def load_xk(b, r, xks):
            xks[b] = []
            for kt in range(KT):
                t_ = xkp.tile([P, kcap], BF16, tag="xk", name=f"xk{r}_{b}_{kt}")
                nc.sync.dma_start(out=t_[:], in_=xk[b, kt * P:(kt + 1) * P, :])
                xks[b].append(t_)

        def load_xt_half(b, half, r, xts):
            if b not in xts:
                xts[b] = [[None, None] for _ in range(KT)]
            c0 = half * QW
            for kt in range(KT):
                t_ = xtp.tile([P, QW], BF16, tag="xt",
                              name=f"xt{r}_{b}_{kt}_{half}")
                nc.sync.dma_start(out=t_[:],
                                  in_=xt[b, kt * P:(kt + 1) * P, c0:c0 + QW])
                xts[b][kt][half] = t_

        def make_proj(b, r):
            qT[b] = qkp.tile([P, N], BF16, tag="qT", name=f"qT{r}_{b}")
            kT[b] = qkp.tile([P, kcap], BF16, tag="kT", name=f"kT{r}_{b}")
            v_sb[b] = vsp.tile([P, NKT, 2, P], BF16, tag="vsb",
                               name=f"vsb{r}_{b}")
            nc.vector.memset(v_sb[b][:, :, :, 64:128], 1.0)

        def q_chunks(b, r, xts, tags):
            th = []

            def q_chunk(c0):
                def run():
                    half, off = c0 // QW, c0 % QW
                    pj = ps.tile([P, 512], FP32, tag=next_tag(tags),
                                 name=f"qp{r}_{b}_{c0}")
                    for kt in range(KT):
                        nc.tensor.matmul(
                            pj[:], wq_sb[:, kt, :],
                            xts[b][kt][half][:, off:off + 512],
                            start=(kt == 0), stop=(kt == KT - 1))
                    nc.vector.tensor_copy(qT[b][:, c0:c0 + 512], pj[:])
                return run

            for c0 in range(0, N, 512):
                th.append(q_chunk(c0))
            return th

        def k_chunks(b, r, xks, tags):
            kcap_b = kcaps[b]
            th = []

            def k_chunk(c0, w):
                def run():
                    pj = ps.tile([P, 512], FP32, tag=next_tag(tags),
                                 name=f"kp{r}_{b}_{c0}")
                    for kt in range(KT):
                        nc.tensor.matmul(
                            pj[:, 0:w], wk_sb[:, kt, :],
                            xks[b][kt][:, c0:c0 + w],
                            start=(kt == 0), stop=(kt == KT - 1))
                    nc.vector.tensor_copy(kT[b][:, c0:c0 + w], pj[:, 0:w])
                return run

            for c0 in range(0, kcap_b, 512):
                th.append(k_chunk(c0, min(512, kcap_b - c0)))
            return th

        def v_chunks(b, r, xks, tags, vgrp=2):
            nkt_b = NKTS[b]
            th = []

            def v_chunk(t0, cnt):
                def run():
                    pj = ps.tile([P, vgrp, P], FP32, tag=next_tag(tags),
                                 name=f"vp{r}_{b}_{t0}")
                    for i in range(cnt):
                        t = t0 + i
                        for kt in range(KT):
                            nc.tensor.matmul(
                                pj[:, i, :],
                                xks[b][kt][:, t * P:(t + 1) * P],
                                wv_sb[:, kt, :],
                                start=(kt == 0), stop=(kt == KT - 1))
                    for i in range(cnt):
                        t = t0 + i
                        nc.vector.tensor_copy(v_sb[b][:, t, 0, 0:64],
                                              pj[:, i, 0:64])
                        nc.vector.tensor_copy(v_sb[b][:, t, 1, 0:64],
                                              pj[:, i, 64:128])
                return run

            for t0 in range(0, nkt_b, vgrp):
                th.append(v_chunk(t0, min(vgrp, nkt_b - t0)))
            return th

        filler = []

        def pop_filler():
            if filler:
                filler.pop(0)()

        def emit_qk(b, jh, c, r, t):
            sA = ps.tile([P, QW], FP32, tag="simA",
                         name=f"sA{r}_{c}{t}")
            sB = ps.tile([P, QW], FP32, tag="simB",
                         name=f"sB{r}_{c}{t}")
            for js in range(2):
                qs = slice(jh * QW + js * 512, jh * QW + (js + 1) * 512)
                ss = slice(js * 512, (js + 1) * 512)
                nc.tensor.matmul(sA[:, ss],
                                 kT[b][0:64, t * P:(t + 1) * P],
                                 qT[b][0:64, qs])
                nc.tensor.matmul(sB[:, ss],
                                 kT[b][64:128, t * P:(t + 1) * P],
                                 qT[b][64:128, qs])
            return sA, sB

        pre_sims = [None]

        def att_block(b, jh, c, r, pop_plan, nxt=None):
            """Attention for query chunk c (=2b+jh): software-pipelined
            t-loop (QK(t+1) emitted between exp(t) and PV(t) so the PE
            dequeues it the moment exp frees the sim bank), then normalize
            (DVE recip+mult), stage, and fire this chunk's AllToAll.  At the
            last iteration the NEXT chunk's QK(0) is emitted (nxt=(b,jh,c))
            so chunk boundaries pipeline the same way."""
            pvA = ps.tile([P, QW], FP32, tag="pvA", name=f"pvA{r}_{c}")
            pvB = ps.tile([P, QW], FP32, tag="pvB", name=f"pvB{r}_{c}")

            sims = pre_sims[0] if pre_sims[0] is not None else \
                emit_qk(b, jh, c, r, 0)
            pre_sims[0] = None
            for t in range(NKTS[b]):
                sA, sB = sims
                pA = pp.tile([P, QW], BF16, tag="pA")
                pB = pp.tile([P, QW], BF16, tag="pB")
                nc.scalar.activation(pA[:], sA[:],
                                     mybir.ActivationFunctionType.Exp,
                                     bias=mb_sb[:, b * NKT + t:b * NKT + t + 1],
                                     scale=SCALE)
                nc.scalar.activation(pB[:], sB[:],
                                     mybir.ActivationFunctionType.Exp,
                                     bias=mb_sb[:, b * NKT + t:b * NKT + t + 1],
                                     scale=SCALE)
                if t + 1 < NKTS[b]:
                    sims = emit_qk(b, jh, c, r, t + 1)
                elif nxt is not None:
                    pre_sims[0] = emit_qk(nxt[0], nxt[1], nxt[2], r, 0)
                st, sp = (t == 0), (t == NKTS[b] - 1)
                for js in range(2):
                    ss = slice(js * 512, (js + 1) * 512)
                    nc.tensor.matmul(pvA[:, ss], v_sb[b][:, t, 0, :],
                                     pA[:, ss], start=st, stop=sp)
                    nc.tensor.matmul(pvB[:, ss], v_sb[b][:, t, 1, :],
                                     pB[:, ss], start=st, stop=sp)
                for _ in range(pop_plan[t] if t < len(pop_plan) else 0):
                    pop_filler()

            attT_blk = atb.tile([P, QW], BF16, tag="attT",
                                name=f"attT{r}_{c}")
            stage_dst = a2a_in[c].rearrange("(d p) f -> p d f", p=P)
            for h, pv in enumerate((pvA, pvB)):
                rc = sml.tile([64, QW], FP32, tag="rc")
                nc.vector.reciprocal(rc[:], pv[64:128, :])
                nc.vector.tensor_mul(attT_blk[64 * h:64 * (h + 1), :],
                                     pv[0:64, :], rc[:])
                nc.gpsimd.dma_start(
                    out=stage_dst[64 * h:64 * (h + 1), :, :],
                    in_=attT_blk[64 * h:64 * (h + 1), :].rearrange(
                        "p (d f) -> p d f", f=P))
            if collective:
                nc.gpsimd.collective_compute(
                    "AllToAll", mybir.AluOpType.bypass,
                    replica_groups=[list(range(NCORES))],
                    ins=[a2a_in[c].opt()], outs=[a2a_out[c].opt()],
                )
            else:  # timing-sim stand-in: local copy of the same volume
                nc.sync.dma_start(out=a2a_out[c][:], in_=a2a_in[c][:])

        a2s = {}

        def ph2_load(c, r):
            """Pull this core's [1024 cols x 128 rows] slab for chunk c into
            SBUF once the chunk's collective has delivered it.  Two half
            DMAs so the out-proj's first contraction blocks can start while
            the second half is still in flight."""
            a2s[c] = a2p.tile([P, KT, P], BF16, tag="a2s", name=f"a2s{r}_{c}")
            src = a2a_out[c].rearrange("(s p) f -> p s f", p=P)
            nc.sync.dma_start(out=a2s[c][:, 0:4, :], in_=src[:, 0:4, :])
            nc.sync.dma_start(out=a2s[c][:, 4:8, :], in_=src[:, 4:8, :])

        def ph2_compute(c, r):
            """Out-projection for chunk c's 128 rows + output store (split
            per 512-wide half so the first store overlaps the second half's
            matmuls)."""
            osb = osp.tile([P, DIM], FP32, tag="outsb", name=f"osb{r}_{c}")
            for js in range(2):
                po = ps.tile([P, 512], FP32,
                             tag=next_tag(("simA", "simB")),
                             name=f"po{r}_{c}_{js}")
                ss = slice(js * 512, (js + 1) * 512)
                for s in range(KT):
                    nc.tensor.matmul(po[:], a2s[c][:, s, :],
                                     wo_sb[:, s, ss],
                                     start=(s == 0), stop=(s == KT - 1))
                nc.vector.tensor_copy(osb[:, ss], po[:])
                nc.scalar.dma_start(
                    out=out[c, :, js * 512:(js + 1) * 512], in_=osb[:, ss])

        def dummies(n, r, tag="pvA"):
            """Dependency-free keep-warm matmuls: bridge PE idle while a
            collective is in flight so the p-state ramp stays at full speed
            (a gap drops following matmuls to the LOW p-state, ~4x slower)."""
            if n <= 0:
                return
            dps = ps.tile([64, 512], FP32, tag=tag, name=f"dummy{r}_{tag}")
            for i in range(n):
                nc.tensor.matmul(dps[:], wz[:, 0:64], wz[:],
                                 start=(i == 0), stop=(i == n - 1))

        PTAGS = ("simA", "simB", "pvA", "pvB")
        for rep in range(reps):
            xts = {}
            xks = {}
            # ---- ordered input stream (sync queue = DMA service order) ----
            load_xk(0, rep, xks)
            load_xt_half(0, 0, rep, xts)
            load_xk(1, rep, xks)
            load_xt_half(0, 1, rep, xts)
            load_xt_half(1, 0, rep, xts)
            if rep == 0:
                nc.sync.dma_start(out=wo_sb[:], in_=wo[:])
            load_xt_half(1, 1, rep, xts)

            # ---- head: k piece 0, q block 0, v piece 0 inline; the rest
            # of batch-0's k/v pieces stream into att chunk 0's t-loop just
            # ahead of the iterations that consume them ----
            make_proj(0, rep)
            for th in k_chunks(0, rep, xks, PTAGS):
                th()
            # pre-att inline projections use the full 4-tag PSUM rotation so
            # a claim never waits on the immediately-preceding piece's DVE
            # copy (2-tag rotation cost ~1.4us here); only in-att fillers
            # must stick to simA/simB
            for th in v_chunks(0, rep, xks, PTAGS):
                th()
            q_chunks(0, rep, xts, PTAGS)[0]()
            q_chunks(0, rep, xts, PTAGS)[1]()
            q0 = q_chunks(0, rep, xts, ("simA", "simB"))
            make_proj(1, rep)
            filler.extend(k_chunks(1, rep, xks, ("simA", "simB")))
            filler.extend(q0[2:4])

            att_block(0, 0, 0, rep, pop_plan=[0, 1, 1, 1, 1, 1, 0, 0, 0],
                      nxt=(0, 1, 1))
            ph2_load(0, rep)

            v1 = v_chunks(1, rep, xks, ("simA", "simB"), vgrp=3)
            q1 = q_chunks(1, rep, xts, ("simA", "simB"))
            # only v1 piece 0 and qT[1] block 0 are needed BEFORE chunk 2
            # starts; v1 pieces 1-2 (key tiles 3..8) pop inside chunk 2
            # ahead of the iterations that consume them, keeping chunk 1
            # lean -- the collective track is saturated c1->c3, so total
            # time tracks chunk 1's end directly
            filler.extend([v1[0], q1[0], q1[1]])
            att_block(0, 1, 1, rep, pop_plan=[0, 1, 1, 1, 0, 0, 0, 0, 0],
                      nxt=(1, 0, 2))
            ph2_load(1, rep)

            filler.extend([v1[1], v1[2], q1[2], q1[3]])
            att_block(1, 0, 2, rep, pop_plan=[1, 1, 1, 1, 0, 0, 0, 0, 0],
                      nxt=(1, 1, 3))
            ph2_load(2, rep)

            att_block(1, 1, 3, rep, pop_plan=[0] * 9)
            ph2_load(3, rep)

            while filler:
                pop_filler()

            ph2_compute(0, rep)
            ph2_compute(1, rep)
            dummies(ND1, rep, tag="pvA")
            ph2_compute(2, rep)
            dummies(ND2, rep, tag="pvB")
            ph2_compute(3, rep)

    nc.finalize()
    return nc


_CACHED = {}
_LAST_KCAP = [(1152, 1152)]


def _get_program(reps=1, kcaps=None):
    if kcaps is None:
        kcaps = _LAST_KCAP[0]
    key = (reps, tuple(kcaps))
    if key not in _CACHED:
        _CACHED[key] = build_program(reps, kcaps=tuple(kcaps))
    return _CACHED[key]


def _kcap_for(mask):
    cnts = np.asarray(mask).sum(axis=1)
    return tuple(max(P, ((int(c) + P - 1) // P) * P) for c in cnts)


def make_in_maps(x, mask, Wq, Wkv, Wo):
    """Host-side shard prep: per-core input dicts (keys compacted by mask)."""
    bf16 = __import__("ml_dtypes").bfloat16
    x = np.asarray(x, dtype=np.float32)
    mask = np.asarray(mask)
    Wq = np.asarray(Wq, dtype=np.float32)
    Wkv = np.asarray(Wkv, dtype=np.float32)
    Wo = np.asarray(Wo, dtype=np.float32)

    kcaps = _kcap_for(mask)
    _LAST_KCAP[0] = kcaps
    kcap = max(kcaps)
    nkt = kcap // P

    xT = np.ascontiguousarray(x.transpose(0, 2, 1)).astype(bf16)  # [B, DIM, N]
    xkT = np.zeros((B, DIM, kcap), dtype=bf16)
    mbias = np.full((B, P, nkt), MASK_NEG, dtype=np.float32)
    for b in range(B):
        idx = np.nonzero(mask[b])[0]
        cnt = len(idx)
        xkT[b, :, :cnt] = xT[b][:, idx]
        valid = (np.arange(kcap) < cnt).reshape(nkt, P).T  # [P, nkt]
        mbias[b][valid] = 0.0
    # [P, B*nkt] flat: one contiguous descriptor per partition
    mbias = np.ascontiguousarray(mbias.transpose(1, 0, 2).reshape(P, B * nkt))

    def wprep(w):  # [DIM, M] -> [P, KT, M] (partition-major, contiguous rows)
        return np.ascontiguousarray(
            w.reshape(KT, P, w.shape[1]).transpose(1, 0, 2)).astype(bf16)

    in_maps = []
    for c in range(NCORES):
        cs = slice(c * CW, (c + 1) * CW)
        in_maps.append({
            "xt": xT,
            "xk": xkT,
            "wq": wprep(Wq[:, cs]),
            "wk": wprep(Wkv[:, cs]),
            "wv": wprep(Wkv[:, DI + c * CW: DI + (c + 1) * CW]),
            "wo": wprep(Wo),
            "mb": mbias,
        })
    return in_maps


def assemble(results):
    # core c holds rows c*128..c*128+128 of every 1024-row chunk (2b+jh)
    full = np.empty((B, N, DIM), dtype=np.float32)
    for c in range(NCORES):
        o = np.asarray(results[c]["out"])  # [NCHUNK, P, DIM]
        for ch in range(NCHUNK):
            b, jh = divmod(ch, NQH)
            r0 = jh * QW + c * P
            full[b, r0:r0 + P, :] = o[ch]
    return full


def kernel(x, mask, Wq, Wkv, Wo):
    from concourse.bass_utils import run_bass_kernel_spmd

    in_maps = make_in_maps(x, mask, Wq, Wkv, Wo)
    nc = _get_program()
    res = run_bass_kernel_spmd(nc, in_maps, list(range(NCORES)))
    return assemble(res.results)
